# revision 41
# baseline (speedup 1.0000x reference)
"""Trainium2 Bass kernel for DepthwiseMultiScaleFIR.

Problem: x[B=4, L=2048, H=16, D=64] fp32, four causal depthwise FIR filters
(k = 3, 7, 15, 31) over the L axis, independent per channel (C = H*D = 1024).
Returns the four filtered outputs, each [B, L, H, D] fp32.

Sharding: channels across 8 NeuronCores (128 channels/core), no cross-core
communication.

Per-core algorithm (all engines):
  The depthwise conv is run on the TensorEngine as a banded block-Toeplitz
  matmul.  Time is tiled in 32-step blocks; 4 channels x 32 time-steps are
  interleaved on the 128 partitions (p = tau*4 + c4).  For each group of 4
  channels and each filter, a pair of 128x128 block-diagonal stationaries
  (intra-block band A, previous-block corner B) is matmul'd against the
  transposed input, accumulating in PSUM:

     out[(to,c4), (b,q)] = sum_ti A[(ti,c4),(to,c4)] * X[(ti,c4), (b,q)]
                         + sum_ti B[(ti,c4),(to,c4)] * X[(ti,c4), (b,q-1)]

  Input/output partition remixes ([c,t] <-> interleaved) are done with PE
  transposes over strided free-dim slices, so every DRAM-side DMA access
  pattern is 3-dim with a contiguous (>=512B) inner run.

  The stationaries are weight-derived; they are pre-packed on the host in
  bf16 (standard weight pre-packing).  The moving operand is cast to bf16
  during PSUM eviction; accumulation stays fp32 in PSUM.
"""

import numpy as np
import ml_dtypes
from contextlib import ExitStack

B, L, H, D = 4, 2048, 16, 64
C = H * D
NCORES = 8
CPC = C // NCORES            # 128 channels per core
KS = (3, 7, 15, 31)
NG = CPC // 4                # 32 groups of 4 channels
S = 32                       # time block size
NQ = L // S                  # 64 blocks per batch
QB = NQ + 1                  # 65 incl. one leading zero-pad block
GCOLS = 2 + B * QB           # 262 X_T cols per group (lead pad + spare)
BT = B * L


def _build_stationaries(w: np.ndarray, k: int) -> np.ndarray:
    """w: [CPC, k] fp32 filter taps -> packed stationaries [128, NG*2*128] bf16.

    Compact form: row p_in = c4*32 + ti, col = (g*2 + ab)*32 + to holds the
    c4's own [32x32] block; expanded on-chip into the block-diagonal
    [128, NG*2*128] stationary. ab=0: band A; ab=1: prev-block corner B.
    """
    ti = np.arange(S)[:, None]
    to = np.arange(S)[None, :]
    d = to - ti
    a_valid = (d >= 0) & (d <= k - 1)
    a_idx = np.where(a_valid, k - 1 - d, 0)
    d2 = d + S
    b_valid = (d2 >= 0) & (d2 <= k - 1)
    b_idx = np.where(b_valid, k - 1 - d2, 0)

    # blocks[c] -> A_c [32,32], B_c [32,32]
    A = np.where(a_valid[None], w[:, a_idx], 0.0)  # [CPC, 32, 32]
    Bm = np.where(b_valid[None], w[:, b_idx], 0.0)

    out = np.zeros((4, S, NG, 2, S), np.float32)  # [c4, ti, g, ab, to] compact
    for g in range(NG):
        for c4 in range(4):
            c = 4 * g + c4
            out[c4, :, g, 0, :] = A[c]
            out[c4, :, g, 1, :] = Bm[c]
    return out.reshape(128, NG * 2 * S).astype(ml_dtypes.bfloat16)


def _emit(tc):
    import concourse.mybir as mybir

    nc = tc.nc
    f32 = mybir.dt.float32
    f32r = mybir.dt.float32r
    bf16 = mybir.dt.bfloat16

    x_d = nc.dram_tensor("x", [B, L, CPC], f32, kind="ExternalInput").ap()
    ident_d = nc.dram_tensor("ident", [128, 128], f32, kind="ExternalInput").ap()
    stat_d = {
        k: nc.dram_tensor(f"stat{k}", [128, NG * 2 * S], bf16,
                          kind="ExternalInput").ap()
        for k in KS
    }
    y_d = {
        k: nc.dram_tensor(f"y{k}", [B, L, CPC], f32, kind="ExternalOutput").ap()
        for k in KS
    }

    with ExitStack() as ctx:
        const = ctx.enter_context(tc.tile_pool(name="const", bufs=1))
        tp_in = ctx.enter_context(tc.tile_pool(name="tp", bufs=4, space="PSUM"))
        convp = ctx.enter_context(tc.tile_pool(name="convp", bufs=4, space="PSUM"))
        tp_out = tp_in
        oto_pool = ctx.enter_context(tc.tile_pool(name="oto", bufs=8))
        oq_pool = ctx.enter_context(tc.tile_pool(name="oq", bufs=2))

        ident = const.tile([128, 128], f32, tag="ident")
        nc.sync.dma_start(ident[:, :], ident_d)

        xq = const.tile([128, 2 * S * CPC], f32, tag="xq", name="xq")
        xq_bf = const.tile([128, 2 * S * CPC], bf16, tag="xqb", name="xqb")
        ident_bf = const.tile([128, 128], bf16, tag="identbf")
        ident_r = const.tile([128, 128], f32, tag="identr")
        # xq[bp][p=(b2,q64), f=(tau*128 + c)]
        x_t = [const.tile([128, GCOLS], bf16, tag=f"xt{g}", name=f"xt{g}")
               for g in range(NG)]
        stat_sb = {k: const.tile([128, NG * 2 * 128], bf16, tag=f"stat{k}",
                              name=f"stat_sb{k}")
                   for k in KS}
        statc_pool = ctx.enter_context(tc.tile_pool(name="statc", bufs=1))

        for ki, k in enumerate(KS):
            statc = statc_pool.tile([128, NG * 2 * S], bf16, tag="statc",
                                    name=f"statc{k}")
            nc.sync.dma_start(statc[:, :], stat_d[k])
            nc.gpsimd.memset(stat_sb[k][:, :], 0.0)
            for c4 in range(4):
                sl = slice(c4 * S, (c4 + 1) * S)
                sview = statc[sl, :].rearrange("p (G t) -> p G t", t=S)
                dview = (stat_sb[k][sl, :]
                         .rearrange("p (G t) -> p G t", t=128)
                         [:, :, c4 * S:(c4 + 1) * S])
                if (ki + c4) % 2 == 0:
                    nc.vector.tensor_copy(dview, sview)
                else:
                    nc.scalar.copy(dview, sview)

        # X_T zero pads: col 0/1 (lead + b0 pad), and col 1 + b*65 for b>=1
        for g in range(NG):
            nc.gpsimd.memset(x_t[g][:, 0:2], 0.0)
            pads = (x_t[g][:, 1:261]
                    .rearrange("p (b q) -> p b q", b=B)[:, 1:, 0:1])
            nc.gpsimd.memset(pads, 0.0)

        # load x: enumeration ((b,q), tau, c); DRAM inner run = 512B.
        # Chunked along tau to align with the bf16 cast chunks, so casts and
        # transposes start as soon as the first 512KB lands.
        x_src = x_d.rearrange("b (q t) c -> (b q) t c", t=S)
        TQ = S // 4
        for j in range(4):
            for bp in (0, 1):
                nc.sync.dma_start(
                    xq[:, :].rearrange("p (w t c) -> p w t c", w=2, c=CPC)
                    [:, bp, j * TQ:(j + 1) * TQ, :],
                    x_src[bp * 128:(bp + 1) * 128, j * TQ:(j + 1) * TQ, :])
        nc.vector.tensor_copy(ident_bf[:, :], ident[:, :])
        nc.vector.tensor_copy(ident_r[:, :].bitcast(f32r), ident[:, :])
        CC = S * CPC // 4
        for h in range(8):
            cview = xq_bf[:, h * CC:(h + 1) * CC]
            sview0 = xq[:, h * CC:(h + 1) * CC]
            if h % 2 == 0:
                nc.vector.tensor_copy(cview, sview0)
            else:
                nc.scalar.copy(cview, sview0)

        # build X_T via PE sub-transposes ([32,128] out at partition 0),
        # evicted with a partition shift into the (c4*32+tau) interleave
        ei = 0
        xv = xq_bf[:, :].rearrange("p (bt c) -> p c bt", c=128)
        for g in range(NG):
            for c4 in range(4):
                t = tp_in.tile([2 * S, 128], bf16, tag="tp", name="tpt")
                nc.tensor.transpose(t[:, :], xv[:, 4 * g + c4, :],
                                    ident_bf[:, :])
                for bp in (0, 1):
                    sview = (t[bp * S:(bp + 1) * S, :]
                             .rearrange("p (b q) -> p b q", b=2))
                    dview = (x_t[g][c4 * S:(c4 + 1) * S,
                                    2 + bp * 2 * QB:2 + (bp + 1) * 2 * QB]
                             .rearrange("p (b q) -> p b q", b=2)[:, :, 0:NQ])
                    if ei % 2 == 0:
                        nc.vector.tensor_copy(dview, sview)
                    else:
                        nc.scalar.copy(dview, sview)
                    ei += 1

        # conv matmuls + output transposes
        eng_i = 0
        for k in KS:
            oq = [oq_pool.tile([128, S * CPC], f32, tag=f"oq{bp}",
                               name=f"oq{k}_{bp}")
                  for bp in (0, 1)]
            for g in range(NG):
                pt = convp.tile([128, B * QB], f32)
                lhs_a = stat_sb[k][:, (2 * g) * 128:(2 * g + 1) * 128]
                lhs_b = stat_sb[k][:, (2 * g + 1) * 128:(2 * g + 2) * 128]
                rhs_a = x_t[g][:, 1:1 + B * QB]
                rhs_b = x_t[g][:, 0:B * QB]
                nc.tensor.matmul(pt[:, :], lhs_a, rhs_a, start=True, stop=False)
                nc.tensor.matmul(pt[:, :], lhs_b, rhs_b, start=False, stop=True)

                oto = oto_pool.tile([128, B * NQ], f32)
                sview = pt[:, :].rearrange("p (b q) -> p b q", b=B)[:, :, 1:QB]
                dview = oto[:, :].rearrange("p (b q) -> p b q", b=B).bitcast(f32r)
                if eng_i % 2 == 0:
                    nc.vector.tensor_copy(dview, sview)
                else:
                    nc.scalar.copy(dview, sview)
                eng_i += 1

                for bp in (0, 1):
                    t2 = tp_out.tile([128, 128], f32, tag="tp", name="tpt2")
                    nc.tensor.transpose(t2[:, :].bitcast(f32r),
                                        oto[:, bp * 128:(bp + 1) * 128]
                                        .bitcast(f32r),
                                        ident_r[:, :].bitcast(f32r))
                    sview2 = t2[:, :].rearrange("p (c t) -> p c t", c=4)
                    dview2 = (oq[bp][:, :]
                              .rearrange("p (t c) -> p c t", c=128)
                              [:, 4 * g:4 * g + 4, :])
                    if eng_i % 2 == 0:
                        nc.vector.tensor_copy(dview2, sview2)
                    else:
                        nc.scalar.copy(dview2, sview2)
                    eng_i += 1

            y_dst = y_d[k].rearrange("b (q t) c -> (b q) t c", t=S)
            for bp in (0, 1):
                nc.sync.dma_start(y_dst[bp * 128:(bp + 1) * 128, :, :],
                                  oq[bp][:, :])


_NC_CACHE: list = [None]


def _get_nc():
    if _NC_CACHE[0] is None:
        import concourse.tile as tile
        from concourse import bacc
        nc = bacc.Bacc("TRN2", target_bir_lowering=False, debug=False,
                       num_devices=NCORES)
        with tile.TileContext(nc) as tc:
            _emit(tc)
        nc.finalize()
        _NC_CACHE[0] = nc
    return _NC_CACHE[0]


def make_in_maps(x, filt3, filt7, filt15, filt31):
    x = np.asarray(x, dtype=np.float32).reshape(B, L, C)
    filts = {3: filt3, 7: filt7, 15: filt15, 31: filt31}
    in_maps = []
    for i in range(NCORES):
        m = {"x": np.ascontiguousarray(x[:, :, i * CPC:(i + 1) * CPC]),
             "ident": np.eye(128, dtype=np.float32)}
        for k in KS:
            w = np.asarray(filts[k], dtype=np.float32).reshape(C, k)
            m[f"stat{k}"] = _build_stationaries(w[i * CPC:(i + 1) * CPC], k)
        in_maps.append(m)
    return in_maps


def assemble_outputs(results):
    outs = []
    for k in KS:
        y = np.empty((B, L, C), np.float32)
        for i in range(NCORES):
            y[:, :, i * CPC:(i + 1) * CPC] = results[i][f"y{k}"]
        outs.append(y.reshape(B, L, H, D))
    return tuple(outs)


def kernel(x, filt3, filt7, filt15, filt31):
    from concourse.bass_utils import run_bass_kernel_spmd
    nc = _get_nc()
    in_maps = make_in_maps(x, filt3, filt7, filt15, filt31)
    res = run_bass_kernel_spmd(nc, in_maps, list(range(NCORES)))
    return assemble_outputs(res.results)
